# revision 26
# baseline (speedup 1.0000x reference)
"""Bilinear RoI pooling kernel for 8x Trainium2 NeuronCores.

Problem: feats (512, 64, 256) f32, boxes (4096, 4) f32 -> out (4096, 512, 7, 7) f32.

Strategy (pure data parallelism over boxes; feats table replicated):
  Host:
    - Build fp16 lookup table T[y*256+x, ct] with channel permutation
      ct -> orig channel 4*(ct%128) + ct//128 (so the final store's DRAM runs
      are 784 B contiguous in the (B, C, 7, 7) output).
    - Per sample (box, i, j): 4 clamped bilinear neighbor row indices and 4
      blend weights (validity masks folded in), mirroring reference math.
  Device (per core: 512 boxes = 25088 samples = 196 blocks of 128):
    - dma_gather (SWDGE): per unit of 7 blocks, 3584 rows of 1 KiB fp16;
      quad-interleaved so slot (bl*4+k) partition s holds neighbor k of
      sample s of block bl.
    - ACT: builds diag(w_k) tiles [128, 128] fp16 from the identity via
      activation(Copy, scale=w_column).
    - PE: per block and 128-channel chunk, a 4-matmul accumulation chain
      into one PSUM region: psum[c, s] = sum_k G_k[s, c] * w_k[s]
      (blend + transpose in one pass; one PSUM bank per chain).
    - DVE: copies PSUM -> SBUF store tiles [128, 4, 392] f32 (8-box groups),
      splitting copies at group boundaries.
    - HWDGE store: 784 B contiguous runs into the output.
"""

import numpy as np

HH, WW = 7, 7
C, Hf, Wf = 512, 64, 256
NROWS = Hf * Wf                   # 16384 table rows
N_CORES = 8
B_TOTAL = 4096
B_CORE = B_TOTAL // N_CORES       # 512
SPB = 128                         # samples per block
UB = 2                            # blocks per gather unit (1024 idx = HW max)
GB = 16                           # boxes per store group
SG = GB * HH * WW                 # 784 samples per store group

_NC_CACHE = {}


def _build_nc(n_blocks):
    """n_blocks must be divisible by UB; n_blocks*128 samples must equal
    n_groups*392 for an integer number of store groups."""
    import concourse.bacc as bacc
    import concourse.mybir as mybir

    n_samples = n_blocks * SPB
    assert n_samples % SG == 0
    units = [(t0, min(UB, n_blocks - t0)) for t0 in range(0, n_blocks, UB)]
    n_units = len(units)
    n_groups = n_samples // SG
    nb_boxes = n_groups * GB

    nc = bacc.Bacc("TRN2", debug=False)
    f16, f32, i16 = mybir.dt.float16, mybir.dt.float32, mybir.dt.int16

    table = nc.dram_tensor("table", [NROWS, C], f16, kind="ExternalInput")
    idx_d = nc.dram_tensor("idx", [128, n_blocks * 32], i16, kind="ExternalInput")
    w_d = nc.dram_tensor("wts", [128, n_blocks * 4], f32, kind="ExternalInput")
    id_d = nc.dram_tensor("ident", [128, 128], f16, kind="ExternalInput")
    out_d = nc.dram_tensor("out", [nb_boxes, C, HH * WW], f32, kind="ExternalOutput")

    idx_sb = nc.alloc_sbuf_tensor("idx_sb", [128, n_blocks * 32], i16)
    w_sb = nc.alloc_sbuf_tensor("w_sb", [128, n_blocks * 4], f32)
    id_sb = nc.alloc_sbuf_tensor("id_sb", [128, 128], f16)
    NBUF = 3
    gt = [nc.alloc_sbuf_tensor(f"gt{i}", [128, 4 * UB, C], f16) for i in range(NBUF)]
    st = [nc.alloc_sbuf_tensor(f"st{i}", [128, 4, SG], f32) for i in range(NBUF)]
    dg = [nc.alloc_sbuf_tensor(f"dg{i}", [128, 128], f16) for i in range(8)]
    ps = [nc.alloc_psum_tensor(f"ps{i}", [128, 512], f32) for i in range(8)]

    io_sem = nc.alloc_semaphore("io_sem")
    gat_sems = [nc.alloc_semaphore(f"gat_sem{i}") for i in range(NBUF)]
    act_sem = nc.alloc_semaphore("act_sem")
    pe_sem = nc.alloc_semaphore("pe_sem")
    cp_sem = nc.alloc_semaphore("cp_sem")
    st_sems = [nc.alloc_semaphore(f"st_sem{i}") for i in range(NBUF)]

    # Copy plan: block t, chunk q -> list of (group, s0, s1) sample sub-ranges.
    def block_ranges(t):
        lo, hi = SPB * t, SPB * (t + 1)
        out = []
        g = lo // SG
        while SG * g < hi:
            out.append((g, max(lo, SG * g), min(hi, SG * (g + 1))))
            g += 1
        return out

    copies_per_block = [4 * len(block_ranges(t)) for t in range(n_blocks)]
    last_block_of_group = [(SG * (g + 1) - 1) // SPB for g in range(n_groups)]

    with nc.Block() as block:

        @block.sync
        def _(sync):
            sync.dma_start(idx_sb[:, :], idx_d[:, :]).then_inc(io_sem, 16)
            sync.dma_start(w_sb[:, :], w_d[:, :]).then_inc(io_sem, 16)
            sync.dma_start(id_sb[:, :], id_d[:, :]).then_inc(io_sem, 16)
            for g in range(n_groups):
                sync.wait_ge(cp_sem, last_block_of_group[g] + 1)
                for q in range(4):
                    dst = out_d[
                        g * GB : (g + 1) * GB, 128 * q : 128 * (q + 1), :
                    ].rearrange("b p r -> p b r")
                    src = st[g % NBUF][:, q, :].rearrange("p (b r) -> p b r", b=GB)
                    sync.dma_start(dst, src).then_inc(st_sems[g % NBUF], 16)
            for i in range(min(NBUF, n_groups)):
                sync.wait_ge(st_sems[i], 64 * ((n_groups - 1 - i) // NBUF + 1))

        @block.gpsimd
        def _(gpsimd):
            gpsimd.wait_ge(io_sem, 48)
            for u, (t0, nb) in enumerate(units):
                if u >= NBUF:
                    # gt[u%NBUF] free once PE finished unit u-NBUF's blocks
                    pt0, pnb = units[u - NBUF]
                    gpsimd.wait_ge(pe_sem, pt0 + pnb)
                nidx = nb * SPB * 4
                gpsimd.dma_gather(
                    gt[u % NBUF][:, 0 : 4 * nb, :],
                    table[:, :],
                    idx_sb[:, t0 * 32 : (t0 + nb) * 32],
                    nidx,
                    nidx,
                    C,
                ).then_inc(gat_sems[u % NBUF], 16)

        @block.scalar
        def _(scalar):
            scalar.wait_ge(io_sem, 48)
            for t in range(n_blocks):
                if t >= 2:
                    scalar.wait_ge(pe_sem, t - 1)  # PE done with block t-2 diags
                last = None
                for k in range(4):
                    last = scalar.activation(
                        dg[(t % 2) * 4 + k][:, :],
                        id_sb[:, :],
                        mybir.ActivationFunctionType.Copy,
                        bias=0.0,
                        scale=w_sb[:, 4 * t + k : 4 * t + k + 1],
                    )
                last.then_inc(act_sem, 1)

        @block.tensor
        def _(tensor):
            for t in range(n_blocks):
                u = t // UB
                if t % UB == 0:
                    tensor.wait_ge(gat_sems[u % NBUF], 16 * (u // NBUF + 1))
                # lhsT slot within the unit's gather tile

                tensor.wait_ge(act_sem, t + 1)
                if t >= 2:
                    tensor.wait_ge(cp_sem, t - 1)  # bank set free (block t-2 copied)
                bl = t % UB
                last = None
                for q in range(4):
                    bank = (t % 2) * 4 + q
                    for k in range(4):
                        last = tensor.matmul(
                            ps[bank][:, 0:128],
                            gt[u % NBUF][:, bl * 4 + k, 128 * q : 128 * (q + 1)],
                            dg[(t % 2) * 4 + k][:, :],
                            start=(k == 0),
                            stop=(k == 3),
                        )
                last.then_inc(pe_sem, 1)

        @block.vector
        def _(vector):
            seen_groups = set()
            for t in range(n_blocks):
                vector.wait_ge(pe_sem, t + 1)
                ranges = block_ranges(t)
                for g, _, _ in ranges:
                    if g not in seen_groups:
                        seen_groups.add(g)
                        if g >= NBUF:
                            vector.wait_ge(st_sems[g % NBUF], 64 * (g // NBUF))
                last = None
                for q in range(4):
                    bank = (t % 2) * 4 + q
                    for g, s0, s1 in ranges:
                        last = vector.tensor_copy(
                            st[g % NBUF][:, q, s0 - SG * g : s1 - SG * g],
                            ps[bank][:, s0 - SPB * t : s1 - SPB * t],
                        )
                last.then_inc(cp_sem, 1)

    nc.compile()
    return nc


def _get_nc(n_blocks):
    if n_blocks not in _NC_CACHE:
        _NC_CACHE[n_blocks] = _build_nc(n_blocks)
    return _NC_CACHE[n_blocks]


def _host_prep(feats, boxes, img_height, img_width):
    """fp16 channel-permuted table + per-sample rows (B,49,4) int32 and
    weights (B,49,4) f32, mirroring the reference math."""
    B = boxes.shape[0]
    T = np.ascontiguousarray(feats.reshape(C, NROWS).T).astype(np.float16)

    f32 = np.float32
    xc, yc, w, h = (boxes[:, k].astype(f32) for k in range(4))
    tx = np.linspace(-1.0, 1.0, WW, dtype=f32)
    ty = np.linspace(-1.0, 1.0, HH, dtype=f32)
    inv_w = f32(1.0) / f32(img_width - 1)
    inv_h = f32(1.0) / f32(img_height - 1)
    gx = (f32(2.0) * xc[:, None] - f32(img_width - 1)) * inv_w \
        + (w * inv_w)[:, None] * tx[None, :]
    gy = (f32(2.0) * yc[:, None] - f32(img_height - 1)) * inv_h \
        + (h * inv_h)[:, None] * ty[None, :]
    px = (gx + f32(1.0)) * f32(0.5) * f32(Wf - 1)   # (B, WW)
    py = (gy + f32(1.0)) * f32(0.5) * f32(Hf - 1)   # (B, HH)

    x0 = np.floor(px)
    y0 = np.floor(py)
    fx, fy = px - x0, py - y0
    x0i, y0i = x0.astype(np.int64), y0.astype(np.int64)
    x1i, y1i = x0i + 1, y0i + 1
    vx0 = ((x0i >= 0) & (x0i <= Wf - 1)).astype(f32)
    vx1 = ((x1i >= 0) & (x1i <= Wf - 1)).astype(f32)
    vy0 = ((y0i >= 0) & (y0i <= Hf - 1)).astype(f32)
    vy1 = ((y1i >= 0) & (y1i <= Hf - 1)).astype(f32)
    x0c = np.clip(x0i, 0, Wf - 1).astype(np.int32)
    x1c = np.clip(x1i, 0, Wf - 1).astype(np.int32)
    y0c = np.clip(y0i, 0, Hf - 1).astype(np.int32)
    y1c = np.clip(y1i, 0, Hf - 1).astype(np.int32)

    def by(a):
        return np.broadcast_to(a[:, :, None], (B, HH, WW))

    def bx(a):
        return np.broadcast_to(a[:, None, :], (B, HH, WW))

    rows = np.stack(
        [
            by(y0c) * Wf + bx(x0c),
            by(y0c) * Wf + bx(x1c),
            by(y1c) * Wf + bx(x0c),
            by(y1c) * Wf + bx(x1c),
        ],
        axis=-1,
    ).reshape(B, HH * WW, 4).astype(np.int32)

    wx0, wx1 = f32(1.0) - fx, fx
    wy0, wy1 = f32(1.0) - fy, fy
    wts = np.stack(
        [
            by(wy0 * vy0) * bx(wx0 * vx0),
            by(wy0 * vy0) * bx(wx1 * vx1),
            by(wy1 * vy1) * bx(wx0 * vx0),
            by(wy1 * vy1) * bx(wx1 * vx1),
        ],
        axis=-1,
    ).reshape(B, HH * WW, 4).astype(f32)
    return T, rows, wts


def _pack_core(rows_c, wts_c):
    """rows_c (nb, 49, 4) int32, wts_c (nb, 49, 4) f32 ->
    idx [128, n_blocks*32] int16 and w [128, n_blocks*4] f32."""
    n_samples = rows_c.shape[0] * HH * WW
    assert n_samples % SPB == 0
    n_blocks = n_samples // SPB

    # Gather order (block-major): i = (bl*4 + k)*128 + s_local within a unit
    # == concatenation of per-block [k, s] runs of 512.
    r = rows_c.reshape(n_blocks, SPB, 4)             # [t, s, k]
    gidx = r.transpose(0, 2, 1).reshape(-1).astype(np.int16)
    # position i -> (partition i%16, column i//16); replicate to 8 core groups
    idx16 = gidx.reshape(-1, 16).T
    idx = np.ascontiguousarray(np.tile(idx16, (8, 1)))

    # Weight columns: w[p, 4t+k] = w_k(sample 128t + p)
    wv = wts_c.reshape(n_blocks, SPB, 4).transpose(1, 0, 2).reshape(SPB, -1)
    return idx, np.ascontiguousarray(wv.astype(np.float32))


def kernel(**inputs):
    from concourse.bass_utils import run_bass_kernel_spmd

    feats = np.asarray(inputs["feats"], dtype=np.float32)
    boxes = np.asarray(inputs["boxes"], dtype=np.float32)
    img_height = int(np.asarray(inputs["img_height"]))
    img_width = int(np.asarray(inputs["img_width"]))

    T, rows, wts = _host_prep(feats, boxes, img_height, img_width)
    ident = np.eye(128, dtype=np.float16)

    n_blocks = B_CORE * HH * WW // SPB  # 196
    nc = _get_nc(n_blocks)
    in_maps = []
    for m in range(N_CORES):
        sl = slice(m * B_CORE, (m + 1) * B_CORE)
        idx, w = _pack_core(rows[sl], wts[sl])
        in_maps.append({"table": T, "idx": idx, "wts": w, "ident": ident})

    res = run_bass_kernel_spmd(nc, in_maps, core_ids=list(range(N_CORES)))
    out = np.concatenate([r["out"] for r in res.results], axis=0)
    return np.ascontiguousarray(out.reshape(B_TOTAL, C, HH, WW))


# revision 32
# speedup vs baseline: 1.4258x; 1.4258x over previous
"""Bilinear RoI pooling kernel for 8x Trainium2 NeuronCores.

Problem: feats (512, 64, 256) f32, boxes (4096, 4) f32 -> out (4096, 512, 7, 7) f32.

Pure data parallelism over boxes; fp16 feats table replicated per core.

Host:
  - fp16 table T[y*256+x, ct], channel-permuted: col ct holds original
    channel 4*(ct%128) + ct//128, so PSUM bank q partition p ends up holding
    channel 4p+q and the store's DRAM runs are 784 B contiguous.
  - Per sample: 4 clamped bilinear neighbor rows + 4 weights (validity
    folded), mirroring the reference math in f32.
Device (per core: 512 boxes = 25088 samples):
  - dma_gather units of 2 blocks (1024 rows of 1 KiB; HW max 1024 descs),
    quad-interleaved: slot (bl*4+k), partition s = neighbor k of sample s.
  - ACT builds diag(w_k) [128, 128] fp16 tiles via activation(Copy, scale).
  - PE: per super-block (8 boxes = 392 samples) and 128-channel chunk, one
    PSUM bank accumulates psum[c, s] = sum_k G_k[s, c] * w_k[s] via 4-matmul
    chains per (block x super) piece at disjoint column ranges.
  - DVE copies [128, 392] PSUM -> b-major store tiles [128, 16, 4, 49] f32.
  - Stores: one DMA per 16-box group, 784 B DRAM runs, alternating between
    the sync and scalar HWDGE rings.
"""

import numpy as np

HH, WW = 7, 7
C, Hf, Wf = 512, 64, 256
NROWS = Hf * Wf                   # 16384 table rows
N_CORES = 8
B_TOTAL = 4096
B_CORE = B_TOTAL // N_CORES       # 512
SPB = 128                         # samples per block
UB = 2                            # blocks per gather unit (1024 idx = HW max)
BSUP = 8                          # boxes per super-block (PSUM region)
SSUP = BSUP * HH * WW             # 392 samples per super-block
GB = 16                           # boxes per store group (= 2 super-blocks)
SG = GB * HH * WW                 # 784
NBUF = 3                          # gather/store buffer depth
DGR = 4                           # diag tile rotation depth (blocks)

_NC_CACHE = {}


def _build_nc(n_blocks):
    import concourse.bacc as bacc
    import concourse.mybir as mybir

    n_samples = n_blocks * SPB
    assert n_samples % SG == 0
    units = [(t0, min(UB, n_blocks - t0)) for t0 in range(0, n_blocks, UB)]
    n_units = len(units)
    n_supers = n_samples // SSUP
    n_groups = n_samples // SG
    nb_boxes = n_groups * GB

    # ---- python-side plans ----
    def unit_of(t):
        return t // UB

    # super s -> list of (block, s0, s1) absolute sample ranges
    def super_items(s):
        lo, hi = SSUP * s, SSUP * (s + 1)
        out = []
        t = lo // SPB
        while SPB * t < hi:
            out.append((t, max(lo, SPB * t), min(hi, SPB * (t + 1))))
            t += 1
        return out

    # last super that consumes block t (for dg / gather-tile reuse)
    def last_super_of_block(t):
        return (SPB * (t + 1) - 1) // SSUP

    def t_max_of_super(s):
        return (SSUP * (s + 1) - 1) // SPB

    nc = bacc.Bacc("TRN2", debug=False)
    f16, f32, i16 = mybir.dt.float16, mybir.dt.float32, mybir.dt.int16

    table = nc.dram_tensor("table", [NROWS, C], f16, kind="ExternalInput")
    idx_d = nc.dram_tensor("idx", [128, n_blocks * 32], i16, kind="ExternalInput")
    w_d = nc.dram_tensor("wts", [128, n_blocks * 4], f32, kind="ExternalInput")
    id_d = nc.dram_tensor("ident", [128, 128], f16, kind="ExternalInput")
    out_d = nc.dram_tensor("out", [nb_boxes, C, HH * WW], f32, kind="ExternalOutput")

    idx_sb = nc.alloc_sbuf_tensor("idx_sb", [128, n_blocks * 32], i16)
    w_sb = nc.alloc_sbuf_tensor("w_sb", [128, n_blocks * 4], f32)
    id_sb = nc.alloc_sbuf_tensor("id_sb", [128, 128], f16)
    gt = [nc.alloc_sbuf_tensor(f"gt{i}", [128, 4 * UB, C], f16) for i in range(NBUF)]
    st = [nc.alloc_sbuf_tensor(f"st{i}", [128, GB, 4, HH * WW], f32) for i in range(NBUF)]
    dg = [nc.alloc_sbuf_tensor(f"dg{i}", [128, 128], f16) for i in range(4 * DGR)]
    ps = [nc.alloc_psum_tensor(f"ps{i}", [128, 512], f32) for i in range(8)]

    io_sem = nc.alloc_semaphore("io_sem")
    gat_sems = [nc.alloc_semaphore(f"gat_sem{i}") for i in range(NBUF)]
    act_sem = nc.alloc_semaphore("act_sem")   # diag build count (per block)
    pe_sem = nc.alloc_semaphore("pe_sem")     # supers completed by PE
    cp_sem = nc.alloc_semaphore("cp_sem")     # supers copied by DVE
    zr_sem = nc.alloc_semaphore("zr_sem")     # supers whose banks are zeroed
    st_sems = [nc.alloc_semaphore(f"st_sem{i}") for i in range(NBUF)]

    # store group g -> issuing engine parity (0 = sync, 1 = scalar)
    def store_engine(g):
        return g % 2

    def emit_store(eng, g):
        eng.wait_ge(cp_sem, 2 * (g + 1))
        dst = out_d[g * GB : (g + 1) * GB].rearrange(
            "b (p j) r -> p b (j r)", p=128, j=4
        )
        src = st[g % NBUF][:, :, :, :].rearrange("p b j r -> p b (j r)")
        eng.dma_start(dst, src).then_inc(st_sems[g % NBUF], 16)

    with nc.Block() as block:

        @block.sync
        def _(sync):
            sync.dma_start(idx_sb[:, :], idx_d[:, :]).then_inc(io_sem, 16)
            sync.dma_start(w_sb[:, :], w_d[:, :]).then_inc(io_sem, 16)
            sync.dma_start(id_sb[:, :], id_d[:, :]).then_inc(io_sem, 16)
            for g in range(n_groups):
                if store_engine(g) == 0:
                    emit_store(sync, g)
            for i in range(min(NBUF, n_groups)):
                sync.wait_ge(st_sems[i], 16 * ((n_groups - 1 - i) // NBUF + 1))

        @block.gpsimd
        def _(gpsimd):
            gpsimd.wait_ge(io_sem, 48)
            for u, (t0, nb) in enumerate(units):
                if u >= NBUF:
                    pt0, pnb = units[u - NBUF]
                    gpsimd.wait_ge(pe_sem, last_super_of_block(pt0 + pnb - 1) + 1)
                nidx = nb * SPB * 4
                gpsimd.dma_gather(
                    gt[u % NBUF][:, 0 : 4 * nb, :],
                    table[:, :],
                    idx_sb[:, t0 * 32 : (t0 + nb) * 32],
                    nidx,
                    nidx,
                    C,
                ).then_inc(gat_sems[u % NBUF], 16)

        @block.scalar
        def _(scalar):
            scalar.wait_ge(io_sem, 48)
            # interleave diag builds (per block) with odd-group stores
            pending = [g for g in range(n_groups) if store_engine(g) == 1]

            def store_release_block(g):
                # emit after diags of this block: by then PE/DVE have reached
                # super 2g+1 comfortably; NBUF store slack absorbs the rest
                return min(t_max_of_super(min(2 * g + 3, n_supers - 1)), n_blocks - 1)

            for t in range(n_blocks):
                if t >= DGR:
                    scalar.wait_ge(pe_sem, last_super_of_block(t - DGR) + 1)
                last = None
                for k in range(4):
                    last = scalar.activation(
                        dg[(t % DGR) * 4 + k][:, :],
                        id_sb[:, :],
                        mybir.ActivationFunctionType.Copy,
                        bias=0.0,
                        scale=w_sb[:, 4 * t + k : 4 * t + k + 1],
                    )
                last.then_inc(act_sem, 1)
                while pending and store_release_block(pending[0]) <= t:
                    emit_store(scalar, pending.pop(0))
            for g in pending:
                emit_store(scalar, g)

        @block.tensor
        def _(tensor):
            seen_units = set()
            for s in range(n_supers):
                items = super_items(s)
                for t, _, _ in items:
                    u = unit_of(t)
                    if u not in seen_units:
                        seen_units.add(u)
                        tensor.wait_ge(gat_sems[u % NBUF], 16 * (u // NBUF + 1))
                tensor.wait_ge(act_sem, items[-1][0] + 1)
                tensor.wait_ge(zr_sem, s + 1)  # bank set (s%2) zeroed
                last = None
                for q in range(4):
                    bank = (s % 2) * 4 + q
                    for t, s0, s1 in items:
                        u, bl = unit_of(t), t % UB
                        o0, o1 = s0 - SSUP * s, s1 - SSUP * s
                        r0, r1 = s0 - SPB * t, s1 - SPB * t
                        for k in range(4):
                            last = tensor.matmul(
                                ps[bank][:, o0:o1],
                                gt[u % NBUF][:, bl * 4 + k, 128 * q : 128 * (q + 1)],
                                dg[(t % DGR) * 4 + k][:, r0:r1],
                                start=False,
                                stop=(k == 3),
                                skip_group_check=True,
                            )
                last.then_inc(pe_sem, 1)

        @block.vector
        def _(vector):
            def zero_banks(sz):
                # banks (sz%2): previous user (super sz-2) already copied out
                # (cp_sem wait is an instantly-satisfied same-engine ordering
                # marker for the race detector)
                if sz >= 2:
                    vector.wait_ge(cp_sem, sz - 1)
                last = None
                for q in range(4):
                    last = vector.memset(ps[(sz % 2) * 4 + q][:, 0:SSUP], 0)
                last.then_inc(zr_sem, 1)

            zero_banks(0)
            if n_supers > 1:
                zero_banks(1)
            for s in range(n_supers):
                g = s // 2
                vector.wait_ge(pe_sem, s + 1)
                if s % 2 == 0 and g >= NBUF:
                    vector.wait_ge(st_sems[g % NBUF], 16 * (g // NBUF))
                last = None
                for q in range(4):
                    bank = (s % 2) * 4 + q
                    last = vector.tensor_copy(
                        st[g % NBUF][:, BSUP * (s % 2) : BSUP * (s % 2 + 1), q, :],
                        ps[bank][:, 0:SSUP].rearrange("p (b r) -> p b r", b=BSUP),
                    )
                last.then_inc(cp_sem, 1)
                if s + 2 < n_supers:
                    zero_banks(s + 2)

    nc.compile()
    return nc


def _get_nc(n_blocks):
    if n_blocks not in _NC_CACHE:
        _NC_CACHE[n_blocks] = _build_nc(n_blocks)
    return _NC_CACHE[n_blocks]


def _host_prep(feats, boxes, img_height, img_width):
    """fp16 channel-permuted table + per-sample rows (B,49,4) int32 and
    weights (B,49,4) f32, mirroring the reference math."""
    B = boxes.shape[0]
    ct = np.arange(C)
    perm = 4 * (ct % 128) + (ct // 128)
    T = feats.reshape(C, NROWS).T
    T = np.ascontiguousarray(T[:, perm]).astype(np.float16)

    f32 = np.float32
    xc, yc, w, h = (boxes[:, k].astype(f32) for k in range(4))
    tx = np.linspace(-1.0, 1.0, WW, dtype=f32)
    ty = np.linspace(-1.0, 1.0, HH, dtype=f32)
    inv_w = f32(1.0) / f32(img_width - 1)
    inv_h = f32(1.0) / f32(img_height - 1)
    gx = (f32(2.0) * xc[:, None] - f32(img_width - 1)) * inv_w \
        + (w * inv_w)[:, None] * tx[None, :]
    gy = (f32(2.0) * yc[:, None] - f32(img_height - 1)) * inv_h \
        + (h * inv_h)[:, None] * ty[None, :]
    px = (gx + f32(1.0)) * f32(0.5) * f32(Wf - 1)   # (B, WW)
    py = (gy + f32(1.0)) * f32(0.5) * f32(Hf - 1)   # (B, HH)

    x0 = np.floor(px)
    y0 = np.floor(py)
    fx, fy = px - x0, py - y0
    x0i, y0i = x0.astype(np.int64), y0.astype(np.int64)
    x1i, y1i = x0i + 1, y0i + 1
    vx0 = ((x0i >= 0) & (x0i <= Wf - 1)).astype(f32)
    vx1 = ((x1i >= 0) & (x1i <= Wf - 1)).astype(f32)
    vy0 = ((y0i >= 0) & (y0i <= Hf - 1)).astype(f32)
    vy1 = ((y1i >= 0) & (y1i <= Hf - 1)).astype(f32)
    x0c = np.clip(x0i, 0, Wf - 1).astype(np.int32)
    x1c = np.clip(x1i, 0, Wf - 1).astype(np.int32)
    y0c = np.clip(y0i, 0, Hf - 1).astype(np.int32)
    y1c = np.clip(y1i, 0, Hf - 1).astype(np.int32)

    def by(a):
        return np.broadcast_to(a[:, :, None], (B, HH, WW))

    def bx(a):
        return np.broadcast_to(a[:, None, :], (B, HH, WW))

    rows = np.stack(
        [
            by(y0c) * Wf + bx(x0c),
            by(y0c) * Wf + bx(x1c),
            by(y1c) * Wf + bx(x0c),
            by(y1c) * Wf + bx(x1c),
        ],
        axis=-1,
    ).reshape(B, HH * WW, 4).astype(np.int32)

    wx0, wx1 = f32(1.0) - fx, fx
    wy0, wy1 = f32(1.0) - fy, fy
    wts = np.stack(
        [
            by(wy0 * vy0) * bx(wx0 * vx0),
            by(wy0 * vy0) * bx(wx1 * vx1),
            by(wy1 * vy1) * bx(wx0 * vx0),
            by(wy1 * vy1) * bx(wx1 * vx1),
        ],
        axis=-1,
    ).reshape(B, HH * WW, 4).astype(f32)
    return T, rows, wts


def _pack_core(rows_c, wts_c):
    """rows_c (nb, 49, 4) int32, wts_c (nb, 49, 4) f32 ->
    idx [128, n_blocks*32] int16 and w [128, n_blocks*4] f32."""
    n_samples = rows_c.shape[0] * HH * WW
    assert n_samples % SPB == 0
    n_blocks = n_samples // SPB

    # Gather order (block-major): i = (bl*4 + k)*128 + s_local
    r = rows_c.reshape(n_blocks, SPB, 4)             # [t, s, k]
    gidx = r.transpose(0, 2, 1).reshape(-1).astype(np.int16)
    idx16 = gidx.reshape(-1, 16).T
    idx = np.ascontiguousarray(np.tile(idx16, (8, 1)))

    # Weight columns: w[p, 4t+k] = w_k(sample 128t + p)
    wv = wts_c.reshape(n_blocks, SPB, 4).transpose(1, 0, 2).reshape(SPB, -1)
    return idx, np.ascontiguousarray(wv.astype(np.float32))


def kernel(**inputs):
    from concourse.bass_utils import run_bass_kernel_spmd

    feats = np.asarray(inputs["feats"], dtype=np.float32)
    boxes = np.asarray(inputs["boxes"], dtype=np.float32)
    img_height = int(np.asarray(inputs["img_height"]))
    img_width = int(np.asarray(inputs["img_width"]))

    T, rows, wts = _host_prep(feats, boxes, img_height, img_width)
    ident = np.eye(128, dtype=np.float16)

    n_blocks = B_CORE * HH * WW // SPB  # 196
    nc = _get_nc(n_blocks)
    in_maps = []
    for m in range(N_CORES):
        sl = slice(m * B_CORE, (m + 1) * B_CORE)
        idx, w = _pack_core(rows[sl], wts[sl])
        in_maps.append({"table": T, "idx": idx, "wts": w, "ident": ident})

    res = run_bass_kernel_spmd(nc, in_maps, core_ids=list(range(N_CORES)))
    out = np.concatenate([r["out"] for r in res.results], axis=0)
    return np.ascontiguousarray(out.reshape(B_TOTAL, C, HH, WW))


# revision 40
# speedup vs baseline: 2.7883x; 1.9556x over previous
"""Bilinear RoI pooling kernel for 8x Trainium2 NeuronCores.

Problem: feats (512, 64, 256) f32, boxes (4096, 4) f32 -> out (4096, 512, 7, 7) f32.

Pure data parallelism over boxes; fp16 feats table replicated per core.

Host:
  - fp16 table T[y*256+x, ct], channel-permuted: col ct holds original
    channel 4*(ct%128) + ct//128, so PSUM bank q partition p ends up holding
    channel 4p+q and the store's DRAM runs are 784 B contiguous.
  - Per sample: 4 clamped bilinear neighbor rows + 4 weights (validity
    folded), mirroring the reference math in f32.
Device (per core: 512 boxes = 25088 samples):
  - dma_gather units of 2 blocks (1024 rows of 1 KiB; HW max 1024 descs),
    quad-interleaved: slot (bl*4+k), partition s = neighbor k of sample s.
  - ACT builds diag(w_k) [128, 128] fp16 tiles via activation(Copy, scale).
  - PE: per super-block (8 boxes = 392 samples) and 128-channel chunk, one
    PSUM bank accumulates psum[c, s] = sum_k G_k[s, c] * w_k[s] via 4-matmul
    chains per (block x super) piece at disjoint column ranges.
  - DVE copies [128, 392] PSUM -> b-major store tiles [128, 16, 4, 49] f32.
  - Stores: one DMA per 16-box group, 784 B DRAM runs, alternating between
    the sync and scalar HWDGE rings.
"""

import numpy as np

HH, WW = 7, 7
C, Hf, Wf = 512, 64, 256
NPY, NPX = Hf - 1, Wf - 1         # patch-base grid 63 x 255
NROWS = NPY * NPX                 # 16065 patch rows
PELEM = 4 * C                     # 2048 fp16 per patch row (tl|tr|bl|br)
N_CORES = 8
B_TOTAL = 4096
B_CORE = B_TOTAL // N_CORES       # 512
SPB = 128                         # samples per block
UB = 4                            # blocks per gather unit (512 patch idx)
BSUP = 8                          # boxes per super-block (PSUM region)
SSUP = BSUP * HH * WW             # 392 samples per super-block
GB = 16                           # boxes per store group (= 2 super-blocks)
SG = GB * HH * WW                 # 784
NBUF = 3                          # gather/store buffer depth
DGR = 4                           # diag tile rotation depth (blocks)

_NC_CACHE = {}


def _build_nc(n_blocks):
    import concourse.bacc as bacc
    import concourse.mybir as mybir

    n_samples = n_blocks * SPB
    assert n_samples % SG == 0
    units = [(t0, min(UB, n_blocks - t0)) for t0 in range(0, n_blocks, UB)]
    n_units = len(units)
    n_supers = n_samples // SSUP
    n_groups = n_samples // SG
    nb_boxes = n_groups * GB

    # ---- python-side plans ----
    def unit_of(t):
        return t // UB

    # super s -> list of (block, s0, s1) absolute sample ranges
    def super_items(s):
        lo, hi = SSUP * s, SSUP * (s + 1)
        out = []
        t = lo // SPB
        while SPB * t < hi:
            out.append((t, max(lo, SPB * t), min(hi, SPB * (t + 1))))
            t += 1
        return out

    # last super that consumes block t (for dg / gather-tile reuse)
    def last_super_of_block(t):
        return (SPB * (t + 1) - 1) // SSUP

    def t_max_of_super(s):
        return (SSUP * (s + 1) - 1) // SPB

    nc = bacc.Bacc("TRN2", debug=False)
    f16, f32, i16 = mybir.dt.float16, mybir.dt.float32, mybir.dt.int16

    table = nc.dram_tensor("table", [NROWS, PELEM], f16, kind="ExternalInput")
    idx_d = nc.dram_tensor("idx", [128, n_blocks * 8], i16, kind="ExternalInput")
    w_d = nc.dram_tensor("wts", [128, n_blocks * 4], f32, kind="ExternalInput")
    id_d = nc.dram_tensor("ident", [128, 128], f16, kind="ExternalInput")
    out_d = nc.dram_tensor("out", [nb_boxes, C, HH * WW], f32, kind="ExternalOutput")

    idx_sb = nc.alloc_sbuf_tensor("idx_sb", [128, n_blocks * 8], i16)
    w_sb = nc.alloc_sbuf_tensor("w_sb", [128, n_blocks * 4], f32)
    id_sb = nc.alloc_sbuf_tensor("id_sb", [128, 128], f16)
    gt = [nc.alloc_sbuf_tensor(f"gt{i}", [128, UB, PELEM], f16) for i in range(NBUF)]
    st = [nc.alloc_sbuf_tensor(f"st{i}", [128, GB, 4, HH * WW], f32) for i in range(NBUF)]
    dg = [nc.alloc_sbuf_tensor(f"dg{i}", [128, 128], f16) for i in range(4 * DGR)]
    ps = [nc.alloc_psum_tensor(f"ps{i}", [128, 512], f32) for i in range(8)]

    io_sem = nc.alloc_semaphore("io_sem")
    gat_sems = [nc.alloc_semaphore(f"gat_sem{i}") for i in range(NBUF)]
    act_sem = nc.alloc_semaphore("act_sem")   # diag build count (per block)
    pe_sem = nc.alloc_semaphore("pe_sem")     # supers completed by PE
    cp_sem = nc.alloc_semaphore("cp_sem")     # supers copied by DVE
    zr_sem = nc.alloc_semaphore("zr_sem")     # supers whose banks are zeroed
    st_sems = [nc.alloc_semaphore(f"st_sem{i}") for i in range(NBUF)]

    # store group g -> issuing engine parity (0 = sync, 1 = scalar)
    def store_engine(g):
        return g % 2

    def emit_store(eng, g):
        eng.wait_ge(cp_sem, 2 * (g + 1))
        dst = out_d[g * GB : (g + 1) * GB].rearrange(
            "b (p j) r -> p b (j r)", p=128, j=4
        )
        src = st[g % NBUF][:, :, :, :].rearrange("p b j r -> p b (j r)")
        eng.dma_start(dst, src).then_inc(st_sems[g % NBUF], 16)

    with nc.Block() as block:

        @block.sync
        def _(sync):
            sync.dma_start(idx_sb[:, :], idx_d[:, :]).then_inc(io_sem, 16)
            sync.dma_start(w_sb[:, :], w_d[:, :]).then_inc(io_sem, 16)
            sync.dma_start(id_sb[:, :], id_d[:, :]).then_inc(io_sem, 16)
            for g in range(n_groups):
                if store_engine(g) == 0:
                    emit_store(sync, g)
            for i in range(min(NBUF, n_groups)):
                sync.wait_ge(st_sems[i], 16 * ((n_groups - 1 - i) // NBUF + 1))

        @block.gpsimd
        def _(gpsimd):
            gpsimd.wait_ge(io_sem, 48)
            for u, (t0, nb) in enumerate(units):
                if u >= NBUF:
                    pt0, pnb = units[u - NBUF]
                    gpsimd.wait_ge(pe_sem, last_super_of_block(pt0 + pnb - 1) + 1)
                nidx = nb * SPB
                gpsimd.dma_gather(
                    gt[u % NBUF][:, 0:nb, :],
                    table[:, :],
                    idx_sb[:, t0 * 8 : (t0 + nb) * 8],
                    nidx,
                    nidx,
                    PELEM,
                ).then_inc(gat_sems[u % NBUF], 16)

        @block.scalar
        def _(scalar):
            scalar.wait_ge(io_sem, 48)
            # interleave diag builds (per block) with odd-group stores
            pending = [g for g in range(n_groups) if store_engine(g) == 1]

            def store_release_block(g):
                # emit after diags of this block: by then PE/DVE have reached
                # super 2g+1 comfortably; NBUF store slack absorbs the rest
                return min(t_max_of_super(min(2 * g + 3, n_supers - 1)), n_blocks - 1)

            for t in range(n_blocks):
                if t >= DGR:
                    scalar.wait_ge(pe_sem, last_super_of_block(t - DGR) + 1)
                last = None
                for k in range(4):
                    last = scalar.activation(
                        dg[(t % DGR) * 4 + k][:, :],
                        id_sb[:, :],
                        mybir.ActivationFunctionType.Copy,
                        bias=0.0,
                        scale=w_sb[:, 4 * t + k : 4 * t + k + 1],
                    )
                last.then_inc(act_sem, 1)
                while pending and store_release_block(pending[0]) <= t:
                    emit_store(scalar, pending.pop(0))
            for g in pending:
                emit_store(scalar, g)

        @block.tensor
        def _(tensor):
            seen_units = set()
            for s in range(n_supers):
                items = super_items(s)
                for t, _, _ in items:
                    u = unit_of(t)
                    if u not in seen_units:
                        seen_units.add(u)
                        tensor.wait_ge(gat_sems[u % NBUF], 16 * (u // NBUF + 1))
                tensor.wait_ge(act_sem, items[-1][0] + 1)
                tensor.wait_ge(zr_sem, s + 1)  # bank set (s%2) zeroed
                last = None
                for q in range(4):
                    bank = (s % 2) * 4 + q
                    for t, s0, s1 in items:
                        u, bl = unit_of(t), t % UB
                        o0, o1 = s0 - SSUP * s, s1 - SSUP * s
                        r0, r1 = s0 - SPB * t, s1 - SPB * t
                        for k in range(4):
                            last = tensor.matmul(
                                ps[bank][:, o0:o1],
                                gt[u % NBUF][
                                    :, bl, 512 * k + 128 * q : 512 * k + 128 * (q + 1)
                                ],
                                dg[(t % DGR) * 4 + k][:, r0:r1],
                                start=False,
                                stop=(k == 3),
                                skip_group_check=True,
                            )
                last.then_inc(pe_sem, 1)

        @block.vector
        def _(vector):
            def zero_banks(sz):
                # banks (sz%2): previous user (super sz-2) already copied out
                # (cp_sem wait is an instantly-satisfied same-engine ordering
                # marker for the race detector)
                if sz >= 2:
                    vector.wait_ge(cp_sem, sz - 1)
                last = None
                for q in range(4):
                    last = vector.memset(ps[(sz % 2) * 4 + q][:, 0:SSUP], 0)
                last.then_inc(zr_sem, 1)

            zero_banks(0)
            if n_supers > 1:
                zero_banks(1)
            for s in range(n_supers):
                g = s // 2
                vector.wait_ge(pe_sem, s + 1)
                if s % 2 == 0 and g >= NBUF:
                    vector.wait_ge(st_sems[g % NBUF], 16 * (g // NBUF))
                last = None
                for q in range(4):
                    bank = (s % 2) * 4 + q
                    last = vector.tensor_copy(
                        st[g % NBUF][:, BSUP * (s % 2) : BSUP * (s % 2 + 1), q, :],
                        ps[bank][:, 0:SSUP].rearrange("p (b r) -> p b r", b=BSUP),
                    )
                last.then_inc(cp_sem, 1)
                if s + 2 < n_supers:
                    zero_banks(s + 2)

    nc.compile()
    return nc


def _get_nc(n_blocks):
    if n_blocks not in _NC_CACHE:
        _NC_CACHE[n_blocks] = _build_nc(n_blocks)
    return _NC_CACHE[n_blocks]


def _host_prep(feats, boxes, img_height, img_width):
    """fp16 channel-permuted 2x2-patch table + per-sample patch rows
    (B,49) int32 and per-slot weights (B,49,4) f32, mirroring the
    reference math."""
    B = boxes.shape[0]
    ct = np.arange(C)
    perm = 4 * (ct % 128) + (ct // 128)
    F = feats.reshape(C, Hf, Wf).transpose(1, 2, 0)[:, :, perm].astype(np.float16)
    # patch row (by, bx) = [F[by,bx] | F[by,bx+1] | F[by+1,bx] | F[by+1,bx+1]]
    T = np.empty((NPY, NPX, 4, C), np.float16)
    T[:, :, 0] = F[:-1, :-1]
    T[:, :, 1] = F[:-1, 1:]
    T[:, :, 2] = F[1:, :-1]
    T[:, :, 3] = F[1:, 1:]
    T = np.ascontiguousarray(T.reshape(NROWS, PELEM))

    f32 = np.float32
    xc, yc, w, h = (boxes[:, k].astype(f32) for k in range(4))
    tx = np.linspace(-1.0, 1.0, WW, dtype=f32)
    ty = np.linspace(-1.0, 1.0, HH, dtype=f32)
    inv_w = f32(1.0) / f32(img_width - 1)
    inv_h = f32(1.0) / f32(img_height - 1)
    gx = (f32(2.0) * xc[:, None] - f32(img_width - 1)) * inv_w \
        + (w * inv_w)[:, None] * tx[None, :]
    gy = (f32(2.0) * yc[:, None] - f32(img_height - 1)) * inv_h \
        + (h * inv_h)[:, None] * ty[None, :]
    px = (gx + f32(1.0)) * f32(0.5) * f32(Wf - 1)   # (B, WW)
    py = (gy + f32(1.0)) * f32(0.5) * f32(Hf - 1)   # (B, HH)

    x0 = np.floor(px)
    y0 = np.floor(py)
    fx, fy = px - x0, py - y0
    x0i, y0i = x0.astype(np.int64), y0.astype(np.int64)
    x1i, y1i = x0i + 1, y0i + 1
    vx0 = ((x0i >= 0) & (x0i <= Wf - 1)).astype(f32)
    vx1 = ((x1i >= 0) & (x1i <= Wf - 1)).astype(f32)
    vy0 = ((y0i >= 0) & (y0i <= Hf - 1)).astype(f32)
    vy1 = ((y1i >= 0) & (y1i <= Hf - 1)).astype(f32)
    x0c = np.clip(x0i, 0, Wf - 1).astype(np.int32)
    x1c = np.clip(x1i, 0, Wf - 1).astype(np.int32)
    y0c = np.clip(y0i, 0, Hf - 1).astype(np.int32)
    y1c = np.clip(y1i, 0, Hf - 1).astype(np.int32)

    def by(a):
        return np.broadcast_to(a[:, :, None], (B, HH, WW))

    def bx(a):
        return np.broadcast_to(a[:, None, :], (B, HH, WW))

    base_y = np.clip(y0i, 0, NPY - 1)                 # (B, HH)
    base_x = np.clip(x0i, 0, NPX - 1)                 # (B, WW)
    rows = (by(base_y) * NPX + bx(base_x)).reshape(B, HH * WW).astype(np.int32)

    wx0, wx1 = f32(1.0) - fx, fx
    wy0, wy1 = f32(1.0) - fy, fy
    wk = np.stack(
        [
            by(wy0 * vy0) * bx(wx0 * vx0),
            by(wy0 * vy0) * bx(wx1 * vx1),
            by(wy1 * vy1) * bx(wx0 * vx0),
            by(wy1 * vy1) * bx(wx1 * vx1),
        ],
        axis=-1,
    ).reshape(B * HH * WW, 4).astype(f32)
    # neighbor k -> patch slot (dy*2 + dx); invalid (w=0) pixels land anywhere
    dy = np.stack(
        [by(y0c - base_y), by(y0c - base_y), by(y1c - base_y), by(y1c - base_y)],
        axis=-1,
    ).reshape(B * HH * WW, 4)
    dx = np.stack(
        [bx(x0c - base_x), bx(x1c - base_x), bx(x0c - base_x), bx(x1c - base_x)],
        axis=-1,
    ).reshape(B * HH * WW, 4)
    slots = np.clip(dy, 0, 1) * 2 + np.clip(dx, 0, 1)
    wts = np.zeros((B * HH * WW, 4), f32)
    np.add.at(wts, (np.arange(B * HH * WW)[:, None], slots), wk)
    return T, rows, wts.reshape(B, HH * WW, 4)


def _pack_core(rows_c, wts_c):
    """rows_c (nb, 49) int32, wts_c (nb, 49, 4) f32 ->
    idx [128, n_blocks*8] int16 and w [128, n_blocks*4] f32."""
    n_samples = rows_c.shape[0] * HH * WW
    assert n_samples % SPB == 0
    n_blocks = n_samples // SPB

    # One patch row per sample, block-major; position i -> (i%16, i//16)
    gidx = rows_c.reshape(-1).astype(np.int16)
    idx16 = gidx.reshape(-1, 16).T
    idx = np.ascontiguousarray(np.tile(idx16, (8, 1)))

    # Weight columns: w[p, 4t+k] = w_k(sample 128t + p)
    wv = wts_c.reshape(n_blocks, SPB, 4).transpose(1, 0, 2).reshape(SPB, -1)
    return idx, np.ascontiguousarray(wv.astype(np.float32))


def kernel(**inputs):
    from concourse.bass_utils import run_bass_kernel_spmd

    feats = np.asarray(inputs["feats"], dtype=np.float32)
    boxes = np.asarray(inputs["boxes"], dtype=np.float32)
    img_height = int(np.asarray(inputs["img_height"]))
    img_width = int(np.asarray(inputs["img_width"]))

    T, rows, wts = _host_prep(feats, boxes, img_height, img_width)
    ident = np.eye(128, dtype=np.float16)

    n_blocks = B_CORE * HH * WW // SPB  # 196
    nc = _get_nc(n_blocks)
    in_maps = []
    for m in range(N_CORES):
        sl = slice(m * B_CORE, (m + 1) * B_CORE)
        idx, w = _pack_core(rows[sl], wts[sl])
        in_maps.append({"table": T, "idx": idx, "wts": w, "ident": ident})

    res = run_bass_kernel_spmd(nc, in_maps, core_ids=list(range(N_CORES)))
    out = np.concatenate([r["out"] for r in res.results], axis=0)
    return np.ascontiguousarray(out.reshape(B_TOTAL, C, HH, WW))


# revision 44
# speedup vs baseline: 3.0921x; 1.1089x over previous
"""Bilinear RoI pooling kernel for 8x Trainium2 NeuronCores.

Problem: feats (512, 64, 256) f32, boxes (4096, 4) f32 -> out (4096, 512, 7, 7) f32.

Pure data parallelism over boxes; fp16 feats table replicated per core.

Host:
  - fp16 table T[y*256+x, ct], channel-permuted: col ct holds original
    channel 4*(ct%128) + ct//128, so PSUM bank q partition p ends up holding
    channel 4p+q and the store's DRAM runs are 784 B contiguous.
  - Per sample: 4 clamped bilinear neighbor rows + 4 weights (validity
    folded), mirroring the reference math in f32.
Device (per core: 512 boxes = 25088 samples):
  - dma_gather units of 2 blocks (1024 rows of 1 KiB; HW max 1024 descs),
    quad-interleaved: slot (bl*4+k), partition s = neighbor k of sample s.
  - ACT builds diag(w_k) [128, 128] fp16 tiles via activation(Copy, scale).
  - PE: per super-block (8 boxes = 392 samples) and 128-channel chunk, one
    PSUM bank accumulates psum[c, s] = sum_k G_k[s, c] * w_k[s] via 4-matmul
    chains per (block x super) piece at disjoint column ranges.
  - DVE copies [128, 392] PSUM -> b-major store tiles [128, 16, 4, 49] f32.
  - Stores: one DMA per 16-box group, 784 B DRAM runs, alternating between
    the sync and scalar HWDGE rings.
"""

import numpy as np

HH, WW = 7, 7
C, Hf, Wf = 512, 64, 256
NPY, NPX = Hf - 1, Wf - 1         # patch-base grid 63 x 255
NROWS = NPY * NPX                 # 16065 patch rows
PELEM = 4 * C                     # 2048 fp16 per patch row (tl|tr|bl|br)
N_CORES = 8
B_TOTAL = 4096
B_CORE = B_TOTAL // N_CORES       # 512
SPB = 128                         # samples per block
UB = 4                            # blocks per gather unit (512 patch idx)
BSUP = 8                          # boxes per super-block (PSUM region)
SSUP = BSUP * HH * WW             # 392 samples per super-block
GB = 16                           # boxes per store group (= 2 super-blocks)
SG = GB * HH * WW                 # 784
NBUF = 4                          # gather buffer depth
STBUF = 3                         # store tile buffer depth
DGR = 8                           # diag tile rotation depth (blocks)

_NC_CACHE = {}


def _build_nc(n_blocks):
    import concourse.bacc as bacc
    import concourse.mybir as mybir

    n_samples = n_blocks * SPB
    assert n_samples % SG == 0
    units = [(t0, min(UB, n_blocks - t0)) for t0 in range(0, n_blocks, UB)]
    n_units = len(units)
    n_supers = n_samples // SSUP
    n_groups = n_samples // SG
    nb_boxes = n_groups * GB

    # ---- python-side plans ----
    def unit_of(t):
        return t // UB

    # super s -> list of (block, s0, s1) absolute sample ranges
    def super_items(s):
        lo, hi = SSUP * s, SSUP * (s + 1)
        out = []
        t = lo // SPB
        while SPB * t < hi:
            out.append((t, max(lo, SPB * t), min(hi, SPB * (t + 1))))
            t += 1
        return out

    # last super that consumes block t (for dg / gather-tile reuse)
    def last_super_of_block(t):
        return (SPB * (t + 1) - 1) // SSUP

    def t_max_of_super(s):
        return (SSUP * (s + 1) - 1) // SPB

    nc = bacc.Bacc("TRN2", debug=False)
    f16, f32, i16 = mybir.dt.float16, mybir.dt.float32, mybir.dt.int16

    table = nc.dram_tensor("table", [NROWS, PELEM], f16, kind="ExternalInput")
    idx_d = nc.dram_tensor("idx", [128, n_blocks * 8], i16, kind="ExternalInput")
    w_d = nc.dram_tensor("wts", [128, n_blocks * 4], f32, kind="ExternalInput")
    id_d = nc.dram_tensor("ident", [128, 128], f16, kind="ExternalInput")
    out_d = nc.dram_tensor("out", [nb_boxes, C, HH * WW], f32, kind="ExternalOutput")

    idx_sb = nc.alloc_sbuf_tensor("idx_sb", [128, n_blocks * 8], i16)
    w_sb = nc.alloc_sbuf_tensor("w_sb", [128, n_blocks * 4], f32)
    id_sb = nc.alloc_sbuf_tensor("id_sb", [128, 128], f16)
    gt = [nc.alloc_sbuf_tensor(f"gt{i}", [128, UB, PELEM], f16) for i in range(NBUF)]
    st = [nc.alloc_sbuf_tensor(f"st{i}", [128, GB, 4, HH * WW], f32) for i in range(STBUF)]
    dg = [nc.alloc_sbuf_tensor(f"dg{i}", [128, 128], f16) for i in range(4 * DGR)]
    ps = [nc.alloc_psum_tensor(f"ps{i}", [128, 512], f32) for i in range(8)]

    io_sem = nc.alloc_semaphore("io_sem")
    zr_sem = nc.alloc_semaphore("zr_sem")     # supers whose banks are zeroed
    gat_sems = [nc.alloc_semaphore(f"gat_sem{i}") for i in range(NBUF)]
    act_sem = nc.alloc_semaphore("act_sem")   # diag build count (per block)
    pe_sem = nc.alloc_semaphore("pe_sem")     # supers completed by PE
    cp_sem = nc.alloc_semaphore("cp_sem")     # supers copied by DVE
    st_sems = [nc.alloc_semaphore(f"st_sem{i}") for i in range(STBUF)]

    # store group g -> issuing engine parity (0 = sync, 1 = scalar)
    def store_engine(g):
        return g % 2

    def emit_store(eng, g):
        eng.wait_ge(cp_sem, 2 * (g + 1))
        dst = out_d[g * GB : (g + 1) * GB].rearrange(
            "b (p j) r -> p b (j r)", p=128, j=4
        )
        src = st[g % STBUF][:, :, :, :].rearrange("p b j r -> p b (j r)")
        eng.dma_start(dst, src).then_inc(st_sems[g % STBUF], 16)

    with nc.Block() as block:

        @block.sync
        def _(sync):
            sync.dma_start(idx_sb[:, :], idx_d[:, :]).then_inc(io_sem, 16)
            sync.dma_start(w_sb[:, :], w_d[:, :]).then_inc(io_sem, 16)
            sync.dma_start(id_sb[:, :], id_d[:, :]).then_inc(io_sem, 16)
            for g in range(n_groups):
                if store_engine(g) == 0:
                    emit_store(sync, g)
            for i in range(min(STBUF, n_groups)):
                sync.wait_ge(st_sems[i], 16 * ((n_groups - 1 - i) // STBUF + 1))

        @block.gpsimd
        def _(gpsimd):
            gpsimd.wait_ge(io_sem, 48)
            for u, (t0, nb) in enumerate(units):
                if u >= NBUF:
                    pt0, pnb = units[u - NBUF]
                    gpsimd.wait_ge(pe_sem, last_super_of_block(pt0 + pnb - 1) + 1)
                nidx = nb * SPB
                gpsimd.dma_gather(
                    gt[u % NBUF][:, 0:nb, :],
                    table[:, :],
                    idx_sb[:, t0 * 8 : (t0 + nb) * 8],
                    nidx,
                    nidx,
                    PELEM,
                ).then_inc(gat_sems[u % NBUF], 16)

        @block.scalar
        def _(scalar):
            scalar.wait_ge(io_sem, 48)
            # interleave diag builds (per block) with odd-group stores
            pending = [g for g in range(n_groups) if store_engine(g) == 1]

            def store_release_block(g):
                # emit after diags of this block: by then PE/DVE have reached
                # super 2g+1 comfortably; NBUF store slack absorbs the rest
                return min(t_max_of_super(min(2 * g + 3, n_supers - 1)), n_blocks - 1)

            for t in range(n_blocks):
                if t >= DGR:
                    scalar.wait_ge(pe_sem, last_super_of_block(t - DGR) + 1)
                last = None
                for k in range(4):
                    last = scalar.activation(
                        dg[(t % DGR) * 4 + k][:, :],
                        id_sb[:, :],
                        mybir.ActivationFunctionType.Copy,
                        bias=0.0,
                        scale=w_sb[:, 4 * t + k : 4 * t + k + 1],
                    )
                last.then_inc(act_sem, 1)
                while pending and store_release_block(pending[0]) <= t:
                    emit_store(scalar, pending.pop(0))
            for g in pending:
                emit_store(scalar, g)

        @block.tensor
        def _(tensor):
            seen_units = set()
            for s in range(n_supers):
                items = super_items(s)
                for t, _, _ in items:
                    u = unit_of(t)
                    if u not in seen_units:
                        seen_units.add(u)
                        tensor.wait_ge(gat_sems[u % NBUF], 16 * (u // NBUF + 1))
                tensor.wait_ge(act_sem, items[-1][0] + 1)
                tensor.wait_ge(zr_sem, s + 1)  # bank set (s%2) zeroed
                last = None
                for q in range(4):
                    bank = (s % 2) * 4 + q
                    for t, s0, s1 in items:
                        u, bl = unit_of(t), t % UB
                        o0, o1 = s0 - SSUP * s, s1 - SSUP * s
                        r0, r1 = s0 - SPB * t, s1 - SPB * t
                        for k in range(4):
                            last = tensor.matmul(
                                ps[bank][:, o0:o1],
                                gt[u % NBUF][
                                    :, bl, 512 * k + 128 * q : 512 * k + 128 * (q + 1)
                                ],
                                dg[(t % DGR) * 4 + k][:, r0:r1],
                                start=False,
                                stop=(k == 3),
                                skip_group_check=True,
                            )
                last.then_inc(pe_sem, 1)

        @block.vector
        def _(vector):
            def zero_banks(sz):
                # banks (sz%2): previous user (super sz-2) already copied out
                # (cp_sem wait is an instantly-satisfied same-engine ordering
                # marker for the race detector)
                if sz >= 2:
                    vector.wait_ge(cp_sem, sz - 1)
                last = None
                for q in range(4):
                    last = vector.memset(ps[(sz % 2) * 4 + q][:, 0:SSUP], 0)
                last.then_inc(zr_sem, 1)

            zero_banks(0)
            if n_supers > 1:
                zero_banks(1)
            for s in range(n_supers):
                g = s // 2
                vector.wait_ge(pe_sem, s + 1)
                if s % 2 == 0 and g >= STBUF:
                    vector.wait_ge(st_sems[g % STBUF], 16 * (g // STBUF))
                last = None
                for q in range(4):
                    bank = (s % 2) * 4 + q
                    last = vector.tensor_copy(
                        st[g % STBUF][:, BSUP * (s % 2) : BSUP * (s % 2 + 1), q, :],
                        ps[bank][:, 0:SSUP].rearrange("p (b r) -> p b r", b=BSUP),
                    )
                last.then_inc(cp_sem, 1)
                if s + 2 < n_supers:
                    zero_banks(s + 2)

    nc.compile()
    return nc


def _get_nc(n_blocks):
    if n_blocks not in _NC_CACHE:
        _NC_CACHE[n_blocks] = _build_nc(n_blocks)
    return _NC_CACHE[n_blocks]


def _host_prep(feats, boxes, img_height, img_width):
    """fp16 channel-permuted 2x2-patch table + per-sample patch rows
    (B,49) int32 and per-slot weights (B,49,4) f32, mirroring the
    reference math."""
    B = boxes.shape[0]
    ct = np.arange(C)
    perm = 4 * (ct % 128) + (ct // 128)
    F = feats.reshape(C, Hf, Wf).transpose(1, 2, 0)[:, :, perm].astype(np.float16)
    # patch row (by, bx) = [F[by,bx] | F[by,bx+1] | F[by+1,bx] | F[by+1,bx+1]]
    T = np.empty((NPY, NPX, 4, C), np.float16)
    T[:, :, 0] = F[:-1, :-1]
    T[:, :, 1] = F[:-1, 1:]
    T[:, :, 2] = F[1:, :-1]
    T[:, :, 3] = F[1:, 1:]
    T = np.ascontiguousarray(T.reshape(NROWS, PELEM))

    f32 = np.float32
    xc, yc, w, h = (boxes[:, k].astype(f32) for k in range(4))
    tx = np.linspace(-1.0, 1.0, WW, dtype=f32)
    ty = np.linspace(-1.0, 1.0, HH, dtype=f32)
    inv_w = f32(1.0) / f32(img_width - 1)
    inv_h = f32(1.0) / f32(img_height - 1)
    gx = (f32(2.0) * xc[:, None] - f32(img_width - 1)) * inv_w \
        + (w * inv_w)[:, None] * tx[None, :]
    gy = (f32(2.0) * yc[:, None] - f32(img_height - 1)) * inv_h \
        + (h * inv_h)[:, None] * ty[None, :]
    px = (gx + f32(1.0)) * f32(0.5) * f32(Wf - 1)   # (B, WW)
    py = (gy + f32(1.0)) * f32(0.5) * f32(Hf - 1)   # (B, HH)

    x0 = np.floor(px)
    y0 = np.floor(py)
    fx, fy = px - x0, py - y0
    x0i, y0i = x0.astype(np.int64), y0.astype(np.int64)
    x1i, y1i = x0i + 1, y0i + 1
    vx0 = ((x0i >= 0) & (x0i <= Wf - 1)).astype(f32)
    vx1 = ((x1i >= 0) & (x1i <= Wf - 1)).astype(f32)
    vy0 = ((y0i >= 0) & (y0i <= Hf - 1)).astype(f32)
    vy1 = ((y1i >= 0) & (y1i <= Hf - 1)).astype(f32)
    x0c = np.clip(x0i, 0, Wf - 1).astype(np.int32)
    x1c = np.clip(x1i, 0, Wf - 1).astype(np.int32)
    y0c = np.clip(y0i, 0, Hf - 1).astype(np.int32)
    y1c = np.clip(y1i, 0, Hf - 1).astype(np.int32)

    def by(a):
        return np.broadcast_to(a[:, :, None], (B, HH, WW))

    def bx(a):
        return np.broadcast_to(a[:, None, :], (B, HH, WW))

    base_y = np.clip(y0i, 0, NPY - 1)                 # (B, HH)
    base_x = np.clip(x0i, 0, NPX - 1)                 # (B, WW)
    rows = (by(base_y) * NPX + bx(base_x)).reshape(B, HH * WW).astype(np.int32)

    wx0, wx1 = f32(1.0) - fx, fx
    wy0, wy1 = f32(1.0) - fy, fy
    wk = np.stack(
        [
            by(wy0 * vy0) * bx(wx0 * vx0),
            by(wy0 * vy0) * bx(wx1 * vx1),
            by(wy1 * vy1) * bx(wx0 * vx0),
            by(wy1 * vy1) * bx(wx1 * vx1),
        ],
        axis=-1,
    ).reshape(B * HH * WW, 4).astype(f32)
    # neighbor k -> patch slot (dy*2 + dx); invalid (w=0) pixels land anywhere
    dy = np.stack(
        [by(y0c - base_y), by(y0c - base_y), by(y1c - base_y), by(y1c - base_y)],
        axis=-1,
    ).reshape(B * HH * WW, 4)
    dx = np.stack(
        [bx(x0c - base_x), bx(x1c - base_x), bx(x0c - base_x), bx(x1c - base_x)],
        axis=-1,
    ).reshape(B * HH * WW, 4)
    slots = np.clip(dy, 0, 1) * 2 + np.clip(dx, 0, 1)
    wts = np.zeros((B * HH * WW, 4), f32)
    np.add.at(wts, (np.arange(B * HH * WW)[:, None], slots), wk)
    return T, rows, wts.reshape(B, HH * WW, 4)


def _pack_core(rows_c, wts_c):
    """rows_c (nb, 49) int32, wts_c (nb, 49, 4) f32 ->
    idx [128, n_blocks*8] int16 and w [128, n_blocks*4] f32."""
    n_samples = rows_c.shape[0] * HH * WW
    assert n_samples % SPB == 0
    n_blocks = n_samples // SPB

    # One patch row per sample, block-major; position i -> (i%16, i//16)
    gidx = rows_c.reshape(-1).astype(np.int16)
    idx16 = gidx.reshape(-1, 16).T
    idx = np.ascontiguousarray(np.tile(idx16, (8, 1)))

    # Weight columns: w[p, 4t+k] = w_k(sample 128t + p)
    wv = wts_c.reshape(n_blocks, SPB, 4).transpose(1, 0, 2).reshape(SPB, -1)
    return idx, np.ascontiguousarray(wv.astype(np.float32))


def kernel(**inputs):
    from concourse.bass_utils import run_bass_kernel_spmd

    feats = np.asarray(inputs["feats"], dtype=np.float32)
    boxes = np.asarray(inputs["boxes"], dtype=np.float32)
    img_height = int(np.asarray(inputs["img_height"]))
    img_width = int(np.asarray(inputs["img_width"]))

    T, rows, wts = _host_prep(feats, boxes, img_height, img_width)
    ident = np.eye(128, dtype=np.float16)

    n_blocks = B_CORE * HH * WW // SPB  # 196
    nc = _get_nc(n_blocks)
    in_maps = []
    for m in range(N_CORES):
        sl = slice(m * B_CORE, (m + 1) * B_CORE)
        idx, w = _pack_core(rows[sl], wts[sl])
        in_maps.append({"table": T, "idx": idx, "wts": w, "ident": ident})

    res = run_bass_kernel_spmd(nc, in_maps, core_ids=list(range(N_CORES)))
    out = np.concatenate([r["out"] for r in res.results], axis=0)
    return np.ascontiguousarray(out.reshape(B_TOTAL, C, HH, WW))
